# revision 7
# baseline (speedup 1.0000x reference)
"""Trainium2 Bass kernel for the CMB power-spectrum emulator problem.

Math: a 4-layer MLP maps phi (512,2) -> diag (128 knots, 512 ch); a natural
cubic spline through the 128 knots is evaluated on a constant 256x256
isotropic-frequency grid, then exp(.)*NORM.

Since the spline is *linear* in the knot values and the evaluation grid is
input-independent, the whole spline stage collapses to one constant matrix
E (65536, 128) precomputed on host:  out = exp(E @ diag + ln NORM).

Device work per core (spatial sharding, 8192 grid points/core, all 512 ch):
  MLP (tiny matmuls) -> diag (128, 512) in SBUF
  for each 128-channel group g and 512-point chunk:
      psum = diag[:, g].T @ ET[:, chunk]        (TensorE, fp32)
      out  = exp(psum + lnNORM)                 (ScalarE LUT, ~2 ULP)
  stores of (128, 4096) fp32 tiles              (DMA, the roofline term)
"""

import numpy as np

B = 512
N_CORES = 8
P_TOTAL = 256 * 256          # spatial points
P_CORE = P_TOTAL // N_CORES  # 8192
NORM = 1.0 / 12661.0

MIN_PHI = np.array([50.0, 0.0075], np.float32)
DPHI = np.array([40.0, 0.0492], np.float32)
MU = np.array([70.0, 0.032], np.float32)
SIG = np.array([20.0, 0.025], np.float32)

_CACHE = {}


def _build_et():
    """Constant ET (128, 65536) fp32 with val_flat = ET.T @ diag."""
    wn = (256.0 * np.fft.fftfreq(256, d=1.0)).reshape(256, 1)
    wn_iso = np.sqrt(wn**2 + wn.reshape(1, 256) ** 2)
    t32 = np.fft.fftshift(wn_iso).diagonal()[128:].astype(np.float32)  # (128,)
    wn_iso32 = wn_iso.astype(np.float32)

    n = 128
    t = t32.astype(np.float64)
    h = np.diff(t)
    A = np.diag(2.0 * (h[:-1] + h[1:])) + np.diag(h[1:-1], 1) + np.diag(h[1:-1], -1)
    D1 = np.zeros((n - 1, n))
    for i in range(n - 1):
        D1[i, i] = -1.0 / h[i]
        D1[i, i + 1] = 1.0 / h[i]
    D2 = 6.0 * (D1[1:] - D1[:-1])
    L = np.zeros((n, n))
    L[1:-1] = np.linalg.solve(A, D2)

    Sa = np.eye(n)[: n - 1]
    Sb = D1 - (h[:, None] / 6.0) * (2.0 * L[:-1] + L[1:])
    Sc = L[:-1] / 2.0
    Sd = (L[1:] - L[:-1]) / (6.0 * h[:, None])

    idx = np.clip(np.searchsorted(t32, wn_iso32, side="right") - 1, 0, n - 2).ravel()
    f = (wn_iso32.ravel() - t32[idx]).astype(np.float64)[:, None]
    E = Sa[idx] + f * (Sb[idx] + f * (Sc[idx] + f * Sd[idx]))  # (65536, 128)
    return np.ascontiguousarray(E.T.astype(np.float32))  # (128, 65536)


def _build_program():
    import concourse.bass as bass
    import concourse.bacc as bacc
    import concourse.mybir as mybir
    from concourse import tile

    f32 = mybir.dt.float32
    nc = bacc.Bacc("TRN2", target_bir_lowering=False, debug=False)

    pht_d = nc.dram_tensor("pht", [2, B], f32, kind="ExternalInput")
    w1_d = nc.dram_tensor("w1", [2, 100], f32, kind="ExternalInput")
    b1_d = nc.dram_tensor("b1", [100, 1], f32, kind="ExternalInput")
    w2_d = nc.dram_tensor("w2", [100, 100], f32, kind="ExternalInput")
    b2_d = nc.dram_tensor("b2", [100, 1], f32, kind="ExternalInput")
    w3_d = nc.dram_tensor("w3", [100, 100], f32, kind="ExternalInput")
    b3_d = nc.dram_tensor("b3", [100, 1], f32, kind="ExternalInput")
    w4_d = nc.dram_tensor("w4", [100, 128], f32, kind="ExternalInput")
    b4_d = nc.dram_tensor("b4", [128, 1], f32, kind="ExternalInput")
    et_d = nc.dram_tensor("et", [128, P_CORE], f32, kind="ExternalInput")
    lnb_d = nc.dram_tensor("lnb", [128, 1], f32, kind="ExternalInput")
    out_d = nc.dram_tensor("out", [B, P_CORE], f32, kind="ExternalOutput")

    ln_norm = float(np.log(np.float64(NORM)))
    Relu = mybir.ActivationFunctionType.Relu
    Ident = mybir.ActivationFunctionType.Identity
    Exp = mybir.ActivationFunctionType.Exp

    N_GRP = 4          # channel groups of 128
    BIG = 4096         # store-tile free size (fp32 -> 2 MiB per store)
    SUB = 512          # matmul / psum free size
    N_BIG = P_CORE // BIG
    N_SUB = BIG // SUB

    with tile.TileContext(nc) as tc:
        with (
            tc.tile_pool(name="const", bufs=1) as cpool,
            tc.tile_pool(name="mlp", bufs=1) as mpool,
            tc.tile_pool(name="stage", bufs=3) as spool,
            tc.tile_pool(name="psum", bufs=4, space=bass.MemorySpace.PSUM) as ppool,
        ):
            # ---- constant loads ----
            et_t = cpool.tile([128, P_CORE], f32, tag="et")
            for q in range(4):
                sl = slice(q * (P_CORE // 4), (q + 1) * (P_CORE // 4))
                nc.sync.dma_start(et_t[:, sl], et_d[:, sl])

            pht_t = cpool.tile([2, B], f32, tag="pht")
            nc.sync.dma_start(pht_t[:], pht_d[:])
            w1_t = cpool.tile([2, 100], f32, tag="w1")
            nc.sync.dma_start(w1_t[:], w1_d[:])
            w2_t = cpool.tile([100, 100], f32, tag="w2")
            nc.sync.dma_start(w2_t[:], w2_d[:])
            w3_t = cpool.tile([100, 100], f32, tag="w3")
            nc.sync.dma_start(w3_t[:], w3_d[:])
            w4_t = cpool.tile([100, 128], f32, tag="w4")
            nc.sync.dma_start(w4_t[:], w4_d[:])
            b1_t = cpool.tile([100, 1], f32, tag="b1")
            nc.sync.dma_start(b1_t[:], b1_d[:])
            b2_t = cpool.tile([100, 1], f32, tag="b2")
            nc.sync.dma_start(b2_t[:], b2_d[:])
            b3_t = cpool.tile([100, 1], f32, tag="b3")
            nc.sync.dma_start(b3_t[:], b3_d[:])
            b4_t = cpool.tile([128, 1], f32, tag="b4")
            nc.sync.dma_start(b4_t[:], b4_d[:])
            lnb_t = cpool.tile([128, 1], f32, tag="lnb")
            nc.sync.dma_start(lnb_t[:], lnb_d[:])

            # ---- MLP (all channels, transposed layout) ----
            p1 = ppool.tile([100, B], f32, tag="ps")
            nc.tensor.matmul(p1[:], w1_t[:], pht_t[:])
            h1 = mpool.tile([100, B], f32, tag="h1")
            nc.scalar.activation(h1[:], p1[:], Relu, bias=b1_t[:])

            p2 = ppool.tile([100, B], f32, tag="ps")
            nc.tensor.matmul(p2[:], w2_t[:], h1[:])
            h2 = mpool.tile([100, B], f32, tag="h2")
            nc.scalar.activation(h2[:], p2[:], Relu, bias=b2_t[:])

            p3 = ppool.tile([100, B], f32, tag="ps")
            nc.tensor.matmul(p3[:], w3_t[:], h2[:])
            h3 = mpool.tile([100, B], f32, tag="h3")
            nc.scalar.activation(h3[:], p3[:], Relu, bias=b3_t[:])

            p4 = ppool.tile([128, B], f32, tag="ps")
            nc.tensor.matmul(p4[:], w4_t[:], h3[:])
            diag = mpool.tile([128, B], f32, tag="diag")
            nc.scalar.activation(diag[:], p4[:], Ident, bias=b4_t[:])

            # ---- main: out[g*128:(g+1)*128, :] = exp(diag_g.T @ ET + lnN) ----
            for g in range(N_GRP):
                lhsT = diag[:, g * 128 : (g + 1) * 128]
                for big in range(N_BIG):
                    stage = spool.tile([128, BIG], f32, tag="stage")
                    for sub in range(N_SUB):
                        off = big * BIG + sub * SUB
                        ps = ppool.tile([128, SUB], f32, tag="ps")
                        nc.tensor.matmul(ps[:], lhsT, et_t[:, off : off + SUB])
                        nc.scalar.activation(
                            stage[:, sub * SUB : (sub + 1) * SUB],
                            ps[:],
                            Exp,
                            bias=lnb_t[:],
                        )
                    nc.sync.dma_start(
                        out_d[g * 128 : (g + 1) * 128, big * BIG : (big + 1) * BIG],
                        stage[:],
                    )

    nc.compile()
    return nc


def _get_cached():
    if "nc" not in _CACHE:
        _CACHE["nc"] = _build_program()
    if "et" not in _CACHE:
        _CACHE["et"] = _build_et()
    return _CACHE["nc"], _CACHE["et"]


def _make_in_maps(phi, W1, b1, W2, b2, W3, b3, W4, b4, ET):
    # fold the input normalization into the first layer
    scale = (DPHI / SIG).astype(np.float32)
    shift = ((MIN_PHI - MU) / SIG).astype(np.float32)
    W1f = (np.asarray(W1, np.float32) * scale[:, None]).astype(np.float32)
    b1f = (np.asarray(b1, np.float32) + shift @ np.asarray(W1, np.float32)).astype(
        np.float32
    )

    common = {
        "pht": np.ascontiguousarray(np.asarray(phi, np.float32).T),
        "w1": np.ascontiguousarray(W1f),
        "b1": np.ascontiguousarray(b1f.reshape(100, 1)),
        "w2": np.ascontiguousarray(np.asarray(W2, np.float32)),
        "b2": np.ascontiguousarray(np.asarray(b2, np.float32).reshape(100, 1)),
        "w3": np.ascontiguousarray(np.asarray(W3, np.float32)),
        "b3": np.ascontiguousarray(np.asarray(b3, np.float32).reshape(100, 1)),
        "w4": np.ascontiguousarray(np.asarray(W4, np.float32)),
        "b4": np.ascontiguousarray(np.asarray(b4, np.float32).reshape(128, 1)),
        "lnb": np.full((128, 1), np.log(np.float64(NORM)), np.float32),
    }
    in_maps = []
    for c in range(N_CORES):
        m = dict(common)
        m["et"] = np.ascontiguousarray(ET[:, c * P_CORE : (c + 1) * P_CORE])
        in_maps.append(m)
    return in_maps


def kernel(phi, W1, b1, W2, b2, W3, b3, W4, b4):
    from concourse.bass_utils import run_bass_kernel_spmd

    nc, ET = _get_cached()
    in_maps = _make_in_maps(phi, W1, b1, W2, b2, W3, b3, W4, b4, ET)
    res = run_bass_kernel_spmd(nc, in_maps, core_ids=list(range(N_CORES)))
    out = np.concatenate([r["out"] for r in res.results], axis=1)  # (512, 65536)
    return np.ascontiguousarray(out.reshape(B, 256, 256))


# revision 18
# speedup vs baseline: 1.1983x; 1.1983x over previous
"""Trainium2 Bass kernel for the CMB power-spectrum emulator problem.

Math: a 4-layer MLP maps phi (512,2) -> diag (128 knots, 512 ch); a natural
cubic spline through the 128 knots is evaluated on a constant 256x256
isotropic-frequency grid, then exp(.)*NORM.

Since the spline is *linear* in the knot values and the evaluation grid is
input-independent, the whole spline stage collapses to one constant matrix
E (65536, 128) precomputed on host:  out = exp(E @ diag + ln NORM).

Device work per core (spatial sharding, 8192 grid points/core, all 512 ch):
  MLP (tiny matmuls) -> diag (128, 512) in SBUF
  for each 128-channel group g and 512-point chunk:
      psum = diag[:, g].T @ ET[:, chunk]        (TensorE, fp32)
      out  = exp(psum + lnNORM)                 (ScalarE LUT, ~2 ULP)
  stores of (128, 4096) fp32 tiles              (DMA, the roofline term)
"""

import numpy as np

B = 512
N_CORES = 8
P_TOTAL = 256 * 256          # spatial points
P_CORE = P_TOTAL // N_CORES  # 8192
NORM = 1.0 / 12661.0

MIN_PHI = np.array([50.0, 0.0075], np.float32)
DPHI = np.array([40.0, 0.0492], np.float32)
MU = np.array([70.0, 0.032], np.float32)
SIG = np.array([20.0, 0.025], np.float32)

_CACHE = {}


def _build_et():
    """Constant ET (128, 65536) fp32 with val_flat = ET.T @ diag."""
    wn = (256.0 * np.fft.fftfreq(256, d=1.0)).reshape(256, 1)
    wn_iso = np.sqrt(wn**2 + wn.reshape(1, 256) ** 2)
    t32 = np.fft.fftshift(wn_iso).diagonal()[128:].astype(np.float32)  # (128,)
    wn_iso32 = wn_iso.astype(np.float32)

    n = 128
    t = t32.astype(np.float64)
    h = np.diff(t)
    A = np.diag(2.0 * (h[:-1] + h[1:])) + np.diag(h[1:-1], 1) + np.diag(h[1:-1], -1)
    D1 = np.zeros((n - 1, n))
    for i in range(n - 1):
        D1[i, i] = -1.0 / h[i]
        D1[i, i + 1] = 1.0 / h[i]
    D2 = 6.0 * (D1[1:] - D1[:-1])
    L = np.zeros((n, n))
    L[1:-1] = np.linalg.solve(A, D2)

    Sa = np.eye(n)[: n - 1]
    Sb = D1 - (h[:, None] / 6.0) * (2.0 * L[:-1] + L[1:])
    Sc = L[:-1] / 2.0
    Sd = (L[1:] - L[:-1]) / (6.0 * h[:, None])

    idx = np.clip(np.searchsorted(t32, wn_iso32, side="right") - 1, 0, n - 2).ravel()
    f = (wn_iso32.ravel() - t32[idx]).astype(np.float64)[:, None]
    E = Sa[idx] + f * (Sb[idx] + f * (Sc[idx] + f * Sd[idx]))  # (65536, 128)
    return np.ascontiguousarray(E.T.astype(np.float32))  # (128, 65536)


import os

# matmul dtype for the big spline matmul: "f32" (4 cyc/row, exact),
# "f32r" (1 cyc/row at N>=256, reduced mantissa), "bf16" (1 cyc/row, 2B)
MODE = os.environ.get("BASS_KERNEL_MODE", "f32r")


def _build_program(mode):
    import concourse.bass as bass
    import concourse.bacc as bacc
    import concourse.mybir as mybir
    from concourse import tile

    f32 = mybir.dt.float32
    et_dt = {
        "bf16": mybir.dt.bfloat16,
        "f32r": mybir.dt.float32r,
        "f32": f32,
    }[mode]
    nc = bacc.Bacc("TRN2", target_bir_lowering=False, debug=False)

    pht_d = nc.dram_tensor("pht", [2, B], f32, kind="ExternalInput")
    w1_d = nc.dram_tensor("w1", [2, 100], f32, kind="ExternalInput")
    b1_d = nc.dram_tensor("b1", [100, 1], f32, kind="ExternalInput")
    w2_d = nc.dram_tensor("w2", [100, 100], f32, kind="ExternalInput")
    b2_d = nc.dram_tensor("b2", [100, 1], f32, kind="ExternalInput")
    w3_d = nc.dram_tensor("w3", [100, 100], f32, kind="ExternalInput")
    b3_d = nc.dram_tensor("b3", [100, 1], f32, kind="ExternalInput")
    w4_d = nc.dram_tensor("w4", [100, 128], f32, kind="ExternalInput")
    b4_d = nc.dram_tensor("b4", [128, 1], f32, kind="ExternalInput")
    et_d = nc.dram_tensor("et", [128, P_CORE], et_dt, kind="ExternalInput")
    lnb_d = nc.dram_tensor("lnb", [128, 1], f32, kind="ExternalInput")
    out_d = nc.dram_tensor("out", [B, P_CORE], f32, kind="ExternalOutput")

    ln_norm = float(np.log(np.float64(NORM)))
    Relu = mybir.ActivationFunctionType.Relu
    Ident = mybir.ActivationFunctionType.Identity
    Exp = mybir.ActivationFunctionType.Exp

    N_GRP = 4          # channel groups of 128
    BIG = 4096         # store-tile free size (fp32 -> 2 MiB per store)
    SUB = 512          # matmul free size (one PSUM bank of fp32)
    PSW = 2048         # psum tile width = 4 banks, one exp per tile
    N_BIG = P_CORE // BIG

    with tile.TileContext(nc) as tc:
        with (
            tc.tile_pool(name="const", bufs=1) as cpool,
            tc.tile_pool(name="mlp", bufs=1) as mpool,
            tc.tile_pool(name="stage", bufs=3) as spool,
            tc.tile_pool(name="psum", bufs=2, space=bass.MemorySpace.PSUM) as ppool,
        ):
            # ---- constant loads (SWDGE: keep the HWDGE queue for stores) ----
            et_t = cpool.tile([128, P_CORE], et_dt, tag="et")
            for q in range(4):
                sl = slice(q * (P_CORE // 4), (q + 1) * (P_CORE // 4))
                nc.gpsimd.dma_start(et_t[:, sl], et_d[:, sl])

            pht_t = cpool.tile([2, B], f32, tag="pht")
            nc.gpsimd.dma_start(pht_t[:], pht_d[:])
            w1_t = cpool.tile([2, 100], f32, tag="w1")
            nc.gpsimd.dma_start(w1_t[:], w1_d[:])
            w2_t = cpool.tile([100, 100], f32, tag="w2")
            nc.gpsimd.dma_start(w2_t[:], w2_d[:])
            w3_t = cpool.tile([100, 100], f32, tag="w3")
            nc.gpsimd.dma_start(w3_t[:], w3_d[:])
            w4_t = cpool.tile([100, 128], f32, tag="w4")
            nc.gpsimd.dma_start(w4_t[:], w4_d[:])
            b1_t = cpool.tile([100, 1], f32, tag="b1")
            nc.gpsimd.dma_start(b1_t[:], b1_d[:])
            b2_t = cpool.tile([100, 1], f32, tag="b2")
            nc.gpsimd.dma_start(b2_t[:], b2_d[:])
            b3_t = cpool.tile([100, 1], f32, tag="b3")
            nc.gpsimd.dma_start(b3_t[:], b3_d[:])
            b4_t = cpool.tile([128, 1], f32, tag="b4")
            nc.gpsimd.dma_start(b4_t[:], b4_d[:])
            lnb_t = cpool.tile([128, 1], f32, tag="lnb")
            nc.gpsimd.dma_start(lnb_t[:], lnb_d[:])

            # ---- MLP (all channels, transposed layout) ----
            p1 = ppool.tile([100, B], f32, tag="ps")
            nc.tensor.matmul(p1[:], w1_t[:], pht_t[:])
            h1 = mpool.tile([100, B], f32, tag="h1")
            nc.scalar.activation(h1[:], p1[:], Relu, bias=b1_t[:])

            p2 = ppool.tile([100, B], f32, tag="ps")
            nc.tensor.matmul(p2[:], w2_t[:], h1[:])
            h2 = mpool.tile([100, B], f32, tag="h2")
            nc.scalar.activation(h2[:], p2[:], Relu, bias=b2_t[:])

            p3 = ppool.tile([100, B], f32, tag="ps")
            nc.tensor.matmul(p3[:], w3_t[:], h2[:])
            h3 = mpool.tile([100, B], f32, tag="h3")
            nc.scalar.activation(h3[:], p3[:], Relu, bias=b3_t[:])

            p4 = ppool.tile([128, B], f32, tag="ps")
            nc.tensor.matmul(p4[:], w4_t[:], h3[:])
            diag = mpool.tile([128, B], et_dt, tag="diag")
            nc.scalar.activation(diag[:], p4[:], Ident, bias=b4_t[:])

            # ---- main: out[g*128:(g+1)*128, :] = exp(diag_g.T @ ET + lnN) ----
            for g in range(N_GRP):
                for big in range(N_BIG):
                    stage = spool.tile([128, BIG], f32, tag="stage")
                    for pj in range(BIG // PSW):
                        ps = ppool.tile([128, PSW], f32, tag="ps")
                        for sub in range(PSW // SUB):
                            off = big * BIG + pj * PSW + sub * SUB
                            nc.tensor.matmul(
                                ps[:, sub * SUB : (sub + 1) * SUB],
                                diag[:, g * 128 : (g + 1) * 128],
                                et_t[:, off : off + SUB],
                            )
                        nc.scalar.activation(
                            stage[:, pj * PSW : (pj + 1) * PSW],
                            ps[:],
                            Exp,
                            bias=lnb_t[:],
                        )
                    nc.sync.dma_start(
                        out_d[g * 128 : (g + 1) * 128, big * BIG : (big + 1) * BIG],
                        stage[:],
                    )

    nc.compile()
    return nc


def _get_cached():
    key = ("nc", MODE)
    if key not in _CACHE:
        _CACHE[key] = _build_program(MODE)
    if "et" not in _CACHE:
        _CACHE["et"] = _build_et()
    return _CACHE[key], _CACHE["et"]


def _make_in_maps(phi, W1, b1, W2, b2, W3, b3, W4, b4, ET):
    # fold the input normalization into the first layer
    scale = (DPHI / SIG).astype(np.float32)
    shift = ((MIN_PHI - MU) / SIG).astype(np.float32)
    W1f = (np.asarray(W1, np.float32) * scale[:, None]).astype(np.float32)
    b1f = (np.asarray(b1, np.float32) + shift @ np.asarray(W1, np.float32)).astype(
        np.float32
    )

    common = {
        "pht": np.ascontiguousarray(np.asarray(phi, np.float32).T),
        "w1": np.ascontiguousarray(W1f),
        "b1": np.ascontiguousarray(b1f.reshape(100, 1)),
        "w2": np.ascontiguousarray(np.asarray(W2, np.float32)),
        "b2": np.ascontiguousarray(np.asarray(b2, np.float32).reshape(100, 1)),
        "w3": np.ascontiguousarray(np.asarray(W3, np.float32)),
        "b3": np.ascontiguousarray(np.asarray(b3, np.float32).reshape(100, 1)),
        "w4": np.ascontiguousarray(np.asarray(W4, np.float32)),
        "b4": np.ascontiguousarray(np.asarray(b4, np.float32).reshape(128, 1)),
        "lnb": np.full((128, 1), np.log(np.float64(NORM)), np.float32),
    }
    in_maps = []
    for c in range(N_CORES):
        m = dict(common)
        shard = np.ascontiguousarray(ET[:, c * P_CORE : (c + 1) * P_CORE])
        if MODE == "bf16":
            import ml_dtypes

            shard = shard.astype(ml_dtypes.bfloat16)
        m["et"] = shard
        in_maps.append(m)
    return in_maps


def kernel(phi, W1, b1, W2, b2, W3, b3, W4, b4):
    from concourse.bass_utils import run_bass_kernel_spmd

    nc, ET = _get_cached()
    in_maps = _make_in_maps(phi, W1, b1, W2, b2, W3, b3, W4, b4, ET)
    res = run_bass_kernel_spmd(nc, in_maps, core_ids=list(range(N_CORES)))
    out = np.concatenate([r["out"] for r in res.results], axis=1)  # (512, 65536)
    return np.ascontiguousarray(out.reshape(B, 256, 256))


# revision 20
# speedup vs baseline: 1.2193x; 1.0175x over previous
"""Trainium2 Bass kernel for the CMB power-spectrum emulator problem.

Math: a 4-layer MLP maps phi (512,2) -> diag (128 knots, 512 ch); a natural
cubic spline through the 128 knots is evaluated on a constant 256x256
isotropic-frequency grid, then exp(.)*NORM.

Since the spline is *linear* in the knot values and the evaluation grid is
input-independent, the whole spline stage collapses to one constant matrix
E (65536, 128) precomputed on host:  out = exp(E @ diag + ln NORM).

Device work per core (spatial sharding, 8192 grid points/core, all 512 ch):
  MLP (tiny f32r matmuls, chunked by 128 channels) -> diag_g (128,128) x4
  for each 128-channel group g and 2048-point psum tile:
      psum[:, k] = diag_g.T @ ET[:, chunk_k]     (TensorE, f32r 1 cyc/row)
      stage      = exp(psum + lnNORM)            (ScalarE LUT, ~2 ULP)
  stores of (128, 4096) fp32 tiles               (DMA, the roofline term)

Queues: params+ET loads ride the ACT HWDGE ring; output stores get the
SP HWDGE ring to themselves.
"""

import os

import numpy as np

B = 512
N_CORES = 8
P_TOTAL = 256 * 256          # spatial points
P_CORE = P_TOTAL // N_CORES  # 8192
NORM = 1.0 / 12661.0

MIN_PHI = np.array([50.0, 0.0075], np.float32)
DPHI = np.array([40.0, 0.0492], np.float32)
MU = np.array([70.0, 0.032], np.float32)
SIG = np.array([20.0, 0.025], np.float32)

# matmul dtype for the spline matmul: "f32" (4 cyc/row, exact),
# "f32r" (1 cyc/row at N>=256, ~19-bit mantissa), "bf16" (1 cyc/row, 2B)
MODE = os.environ.get("BASS_KERNEL_MODE", "f32r")

# packed-parameter column layout (partition dim x columns), fp32r part
_PM_PHT = slice(0, 512)
_PM_W1 = slice(512, 612)
_PM_W2 = slice(612, 712)
_PM_W3 = slice(712, 812)
_PM_W4 = slice(812, 940)
PM_COLS = 940
# fp32 part: biases + exp bias column
PB_COLS = 5  # b1, b2, b3, b4, ln(NORM)

_CACHE = {}


def _build_et():
    """Constant ET (128, 65536) fp32 with val_flat = ET.T @ diag."""
    wn = (256.0 * np.fft.fftfreq(256, d=1.0)).reshape(256, 1)
    wn_iso = np.sqrt(wn**2 + wn.reshape(1, 256) ** 2)
    t32 = np.fft.fftshift(wn_iso).diagonal()[128:].astype(np.float32)  # (128,)
    wn_iso32 = wn_iso.astype(np.float32)

    n = 128
    t = t32.astype(np.float64)
    h = np.diff(t)
    A = np.diag(2.0 * (h[:-1] + h[1:])) + np.diag(h[1:-1], 1) + np.diag(h[1:-1], -1)
    D1 = np.zeros((n - 1, n))
    for i in range(n - 1):
        D1[i, i] = -1.0 / h[i]
        D1[i, i + 1] = 1.0 / h[i]
    D2 = 6.0 * (D1[1:] - D1[:-1])
    L = np.zeros((n, n))
    L[1:-1] = np.linalg.solve(A, D2)

    Sa = np.eye(n)[: n - 1]
    Sb = D1 - (h[:, None] / 6.0) * (2.0 * L[:-1] + L[1:])
    Sc = L[:-1] / 2.0
    Sd = (L[1:] - L[:-1]) / (6.0 * h[:, None])

    idx = np.clip(np.searchsorted(t32, wn_iso32, side="right") - 1, 0, n - 2).ravel()
    f = (wn_iso32.ravel() - t32[idx]).astype(np.float64)[:, None]
    E = Sa[idx] + f * (Sb[idx] + f * (Sc[idx] + f * Sd[idx]))  # (65536, 128)
    return np.ascontiguousarray(E.T.astype(np.float32))  # (128, 65536)


def _build_program(mode):
    import concourse.bass as bass
    import concourse.bacc as bacc
    import concourse.mybir as mybir
    from concourse import tile

    f32 = mybir.dt.float32
    mm_dt = {
        "bf16": mybir.dt.bfloat16,
        "f32r": mybir.dt.float32r,
        "f32": f32,
    }[mode]
    nc = bacc.Bacc("TRN2", target_bir_lowering=False, debug=False)

    pm_d = nc.dram_tensor("pm", [128, PM_COLS], mm_dt, kind="ExternalInput")
    pb_d = nc.dram_tensor("pb", [128, PB_COLS], f32, kind="ExternalInput")
    ET_CH = 2048  # one et DMA chunk
    N_ETCH = P_CORE // ET_CH
    et_d = [
        nc.dram_tensor(f"et{q}", [128, ET_CH], mm_dt, kind="ExternalInput")
        for q in range(N_ETCH)
    ]
    out_d = nc.dram_tensor("out", [B, P_CORE], f32, kind="ExternalOutput")

    Relu = mybir.ActivationFunctionType.Relu
    Ident = mybir.ActivationFunctionType.Identity
    Exp = mybir.ActivationFunctionType.Exp

    N_GRP = 4          # channel groups of 128
    BIG = 4096         # store-tile free size (fp32 -> 2 MiB per store)
    SUB = 512          # matmul free size (one PSUM bank of fp32)
    PSW = 2048         # psum tile width = 4 banks, one exp per tile
    N_BIG = P_CORE // BIG

    with tile.TileContext(nc) as tc:
        with (
            tc.tile_pool(name="const", bufs=1) as cpool,
            tc.tile_pool(name="mlp", bufs=2) as mpool,
            tc.tile_pool(name="stage", bufs=3) as spool,
            tc.tile_pool(name="psum", bufs=2, space=bass.MemorySpace.PSUM) as ppool,
        ):
            # ---- loads: params first (MLP-critical), then ET chunks ----
            pm_t = cpool.tile([128, PM_COLS], mm_dt, tag="pm")
            nc.scalar.dma_start(pm_t[:], pm_d[:])
            pb_t = cpool.tile([128, PB_COLS], f32, tag="pb")
            nc.scalar.dma_start(pb_t[:], pb_d[:])
            et_t = []
            for q in range(N_ETCH):
                t = cpool.tile([128, ET_CH], mm_dt, tag=f"et{q}")
                nc.scalar.dma_start(t[:], et_d[q][:])
                et_t.append(t)

            pht = pm_t[0:2, _PM_PHT]
            w1 = pm_t[0:2, _PM_W1]
            w2 = pm_t[0:100, _PM_W2]
            w3 = pm_t[0:100, _PM_W3]
            w4 = pm_t[0:100, _PM_W4]
            b1 = pb_t[0:100, 0:1]
            b2 = pb_t[0:100, 1:2]
            b3 = pb_t[0:100, 2:3]
            b4 = pb_t[0:128, 3:4]
            lnb = pb_t[0:128, 4:5]

            # ---- MLP in 4 channel-chunks so diag_0 lands early ----
            diag_g = []
            for c in range(N_GRP):
                cs = slice(c * 128, (c + 1) * 128)
                ps1 = ppool.tile([128, PSW], f32, tag="ps")
                nc.tensor.matmul(ps1[0:100, 0:128], w1, pht[:, cs])
                h1 = mpool.tile([100, 128], mm_dt, tag="h1")
                nc.scalar.activation(h1[:], ps1[0:100, 0:128], Relu, bias=b1)

                ps2 = ppool.tile([128, PSW], f32, tag="ps")
                nc.tensor.matmul(ps2[0:100, 0:128], w2, h1[:])
                h2 = mpool.tile([100, 128], mm_dt, tag="h2")
                nc.scalar.activation(h2[:], ps2[0:100, 0:128], Relu, bias=b2)

                ps3 = ppool.tile([128, PSW], f32, tag="ps")
                nc.tensor.matmul(ps3[0:100, 0:128], w3, h2[:])
                h3 = mpool.tile([100, 128], mm_dt, tag="h3")
                nc.scalar.activation(h3[:], ps3[0:100, 0:128], Relu, bias=b3)

                ps4 = ppool.tile([128, PSW], f32, tag="ps")
                nc.tensor.matmul(ps4[0:128, 0:128], w4, h3[:])
                dg = cpool.tile([128, 128], mm_dt, tag=f"diag{c}")
                nc.scalar.activation(dg[:], ps4[0:128, 0:128], Ident, bias=b4)
                diag_g.append(dg)

            # ---- main: out[g*128:(g+1)*128, :] = exp(diag_g.T @ ET + lnN) ----
            for g in range(N_GRP):
                for big in range(N_BIG):
                    stage = spool.tile([128, BIG], f32, tag="stage")
                    for pj in range(BIG // PSW):
                        ps = ppool.tile([128, PSW], f32, tag="ps")
                        base = big * BIG + pj * PSW
                        for sub in range(PSW // SUB):
                            off = base + sub * SUB
                            nc.tensor.matmul(
                                ps[:, sub * SUB : (sub + 1) * SUB],
                                diag_g[g][:],
                                et_t[off // ET_CH][:, off % ET_CH : off % ET_CH + SUB],
                            )
                        nc.scalar.activation(
                            stage[:, pj * PSW : (pj + 1) * PSW],
                            ps[:],
                            Exp,
                            bias=lnb,
                        )
                    nc.sync.dma_start(
                        out_d[g * 128 : (g + 1) * 128, big * BIG : (big + 1) * BIG],
                        stage[:],
                    )

    nc.compile()
    return nc


def _get_cached():
    key = ("nc", MODE)
    if key not in _CACHE:
        _CACHE[key] = _build_program(MODE)
    if "et" not in _CACHE:
        _CACHE["et"] = _build_et()
    return _CACHE[key], _CACHE["et"]


def _np_mm_dtype():
    if MODE == "bf16":
        import ml_dtypes

        return ml_dtypes.bfloat16
    return np.float32


def _make_in_maps(phi, W1, b1, W2, b2, W3, b3, W4, b4, ET):
    mmdt = _np_mm_dtype()
    # fold the input normalization into the first layer
    scale = (DPHI / SIG).astype(np.float32)
    shift = ((MIN_PHI - MU) / SIG).astype(np.float32)
    W1f = (np.asarray(W1, np.float32) * scale[:, None]).astype(np.float32)
    b1f = (np.asarray(b1, np.float32) + shift @ np.asarray(W1, np.float32)).astype(
        np.float32
    )

    pm = np.zeros((128, PM_COLS), np.float32)
    pm[0:2, _PM_PHT] = np.asarray(phi, np.float32).T
    pm[0:2, _PM_W1] = W1f
    pm[0:100, _PM_W2] = np.asarray(W2, np.float32)
    pm[0:100, _PM_W3] = np.asarray(W3, np.float32)
    pm[0:100, _PM_W4] = np.asarray(W4, np.float32)
    pb = np.zeros((128, PB_COLS), np.float32)
    pb[0:100, 0] = np.asarray(b1f, np.float32)
    pb[0:100, 1] = np.asarray(b2, np.float32)
    pb[0:100, 2] = np.asarray(b3, np.float32)
    pb[0:128, 3] = np.asarray(b4, np.float32)
    pb[:, 4] = np.log(np.float64(NORM))

    common = {"pm": pm.astype(mmdt), "pb": pb}
    in_maps = []
    ET_CH = 2048
    for c in range(N_CORES):
        m = dict(common)
        shard = ET[:, c * P_CORE : (c + 1) * P_CORE]
        for q in range(P_CORE // ET_CH):
            m[f"et{q}"] = np.ascontiguousarray(
                shard[:, q * ET_CH : (q + 1) * ET_CH]
            ).astype(mmdt)
        in_maps.append(m)
    return in_maps


def kernel(phi, W1, b1, W2, b2, W3, b3, W4, b4):
    from concourse.bass_utils import run_bass_kernel_spmd

    nc, ET = _get_cached()
    in_maps = _make_in_maps(phi, W1, b1, W2, b2, W3, b3, W4, b4, ET)
    res = run_bass_kernel_spmd(nc, in_maps, core_ids=list(range(N_CORES)))
    out = np.concatenate([r["out"] for r in res.results], axis=1)  # (512, 65536)
    return np.ascontiguousarray(out.reshape(B, 256, 256))


# revision 21
# speedup vs baseline: 2.6526x; 2.1755x over previous
"""Trainium2 Bass kernel for the CMB power-spectrum emulator problem.

Math: a 4-layer MLP maps phi (512,2) -> diag (128 knots, 512 ch); a natural
cubic spline through the 128 knots is evaluated on a constant 256x256
isotropic-frequency grid, then exp(.)*NORM.

Two structural collapses, both input-independent:
 1. The spline is linear in the knot values, so the whole spline stage is
    one constant matrix E:  out = exp(E @ diag + ln NORM).
 2. The grid value wn_iso[i,j] depends only on (a,b) = sorted(|wn_i|,|wn_j|),
    an exact 8-fold dihedral symmetry: only 8385 of the 65536 grid points
    are distinct, and equal points produce bitwise-equal outputs. The device
    computes the 8385 unique points; the host replicates them with a
    constant gather.

Device work per core (unique-point sharding, 1056 points/core, 512 ch):
  MLP (tiny f32r matmuls, chunked by 128 channels) -> diag_g (128,128) x4
  per 128-channel group g: psum = diag_g.T @ ET_u  (TensorE, f32r)
                           stage = exp(psum+lnN)   (ScalarE LUT, ~2 ULP)
                           store (128, 1056) fp32
"""

import os

import numpy as np

B = 512
N_CORES = 8
N_UNIQ = 129 * 130 // 2       # 8385 distinct grid values
P_CORE = 1056                 # per-core unique points (8 x 1056 = 8448 padded)
P_PAD = N_CORES * P_CORE
NORM = 1.0 / 12661.0

MIN_PHI = np.array([50.0, 0.0075], np.float32)
DPHI = np.array([40.0, 0.0492], np.float32)
MU = np.array([70.0, 0.032], np.float32)
SIG = np.array([20.0, 0.025], np.float32)

# matmul dtype: "f32" (4 cyc/row, exact), "f32r" (1 cyc/row, ~19-bit mantissa)
MODE = os.environ.get("BASS_KERNEL_MODE", "f32r")

# packed-parameter column layout (partition dim x columns), matmul-dtype part
_PM_PHT = slice(0, 512)
_PM_W1 = slice(512, 612)
_PM_W2 = slice(612, 712)
_PM_W3 = slice(712, 812)
_PM_W4 = slice(812, 940)
PM_COLS = 940
PB_COLS = 5  # fp32 part: b1, b2, b3, b4, ln(NORM)

_CACHE = {}


def _spline_eval_matrix(wn_vals):
    """E (len(wn_vals), 128) fp32: natural-cubic-spline evaluation at wn_vals,
    linear in the 128 knot values (knots t_k = sqrt(2)*k in fp32)."""
    wn = (256.0 * np.fft.fftfreq(256, d=1.0)).reshape(256, 1)
    wn_iso = np.sqrt(wn**2 + wn.reshape(1, 256) ** 2)
    t32 = np.fft.fftshift(wn_iso).diagonal()[128:].astype(np.float32)  # (128,)

    n = 128
    t = t32.astype(np.float64)
    h = np.diff(t)
    A = np.diag(2.0 * (h[:-1] + h[1:])) + np.diag(h[1:-1], 1) + np.diag(h[1:-1], -1)
    D1 = np.zeros((n - 1, n))
    for i in range(n - 1):
        D1[i, i] = -1.0 / h[i]
        D1[i, i + 1] = 1.0 / h[i]
    D2 = 6.0 * (D1[1:] - D1[:-1])
    L = np.zeros((n, n))
    L[1:-1] = np.linalg.solve(A, D2)

    Sa = np.eye(n)[: n - 1]
    Sb = D1 - (h[:, None] / 6.0) * (2.0 * L[:-1] + L[1:])
    Sc = L[:-1] / 2.0
    Sd = (L[1:] - L[:-1]) / (6.0 * h[:, None])

    w32 = wn_vals.astype(np.float32)
    idx = np.clip(np.searchsorted(t32, w32, side="right") - 1, 0, n - 2)
    f = (w32 - t32[idx]).astype(np.float64)[:, None]
    E = Sa[idx] + f * (Sb[idx] + f * (Sc[idx] + f * Sd[idx]))
    return E.astype(np.float32)


def _build_constants():
    """ET_u (128, P_PAD) fp32 for the unique points, and IDX (65536,) int32
    mapping each full-grid point to its unique-point column."""
    k = np.arange(256)
    absw = np.minimum(k, 256 - k)  # |wn_i|, with |wn_0| = 0, |wn_128| = 128
    ai = np.minimum(absw[:, None], absw[None, :])
    bi = np.maximum(absw[:, None], absw[None, :])
    uid = (bi * (bi + 1)) // 2 + ai  # (256,256) in [0, N_UNIQ)

    bs = np.concatenate([np.full(b + 1, b) for b in range(129)])  # uid -> b
    as_ = np.concatenate([np.arange(b + 1) for b in range(129)])  # uid -> a
    wn_vals = np.sqrt((as_.astype(np.float64)) ** 2 + (bs.astype(np.float64)) ** 2)

    E = _spline_eval_matrix(wn_vals)  # (8385, 128)
    ET = np.zeros((128, P_PAD), np.float32)
    ET[:, :N_UNIQ] = E.T
    return np.ascontiguousarray(ET), uid.ravel().astype(np.int32)


def _build_program(mode):
    import concourse.bass as bass
    import concourse.bacc as bacc
    import concourse.mybir as mybir
    from concourse import tile

    f32 = mybir.dt.float32
    mm_dt = {"f32r": mybir.dt.float32r, "f32": f32}[mode]
    nc = bacc.Bacc("TRN2", target_bir_lowering=False, debug=False)

    pm_d = nc.dram_tensor("pm", [128, PM_COLS], mm_dt, kind="ExternalInput")
    pb_d = nc.dram_tensor("pb", [128, PB_COLS], f32, kind="ExternalInput")
    et_d = nc.dram_tensor("et", [128, P_CORE], mm_dt, kind="ExternalInput")
    out_d = nc.dram_tensor("out", [B, P_CORE], f32, kind="ExternalOutput")

    Relu = mybir.ActivationFunctionType.Relu
    Ident = mybir.ActivationFunctionType.Identity
    Exp = mybir.ActivationFunctionType.Exp

    N_GRP = 4
    SUB = 512  # matmul free chunk (PSUM bank)

    with tile.TileContext(nc) as tc:
        with (
            tc.tile_pool(name="const", bufs=1) as cpool,
            tc.tile_pool(name="mlp", bufs=2) as mpool,
            tc.tile_pool(name="stage", bufs=2) as spool,
            tc.tile_pool(name="psum", bufs=2, space=bass.MemorySpace.PSUM) as ppool,
        ):
            # ---- loads: params first (MLP-critical), then unique-point ET ----
            pm_t = cpool.tile([128, PM_COLS], mm_dt, tag="pm")
            nc.scalar.dma_start(pm_t[:], pm_d[:])
            pb_t = cpool.tile([128, PB_COLS], f32, tag="pb")
            nc.scalar.dma_start(pb_t[:], pb_d[:])
            et_t = cpool.tile([128, P_CORE], mm_dt, tag="et")
            nc.scalar.dma_start(et_t[:], et_d[:])

            pht = pm_t[0:2, _PM_PHT]
            w1 = pm_t[0:2, _PM_W1]
            w2 = pm_t[0:100, _PM_W2]
            w3 = pm_t[0:100, _PM_W3]
            w4 = pm_t[0:100, _PM_W4]
            b1 = pb_t[0:100, 0:1]
            b2 = pb_t[0:100, 1:2]
            b3 = pb_t[0:100, 2:3]
            b4 = pb_t[0:128, 3:4]
            lnb = pb_t[0:128, 4:5]

            # ---- MLP in 4 channel-chunks so diag_0 lands early ----
            diag_g = []
            for c in range(N_GRP):
                cs = slice(c * 128, (c + 1) * 128)
                ps1 = ppool.tile([128, P_CORE], f32, tag="ps")
                nc.tensor.matmul(ps1[0:100, 0:128], w1, pht[:, cs])
                h1 = mpool.tile([100, 128], mm_dt, tag="h1")
                nc.scalar.activation(h1[:], ps1[0:100, 0:128], Relu, bias=b1)

                ps2 = ppool.tile([128, P_CORE], f32, tag="ps")
                nc.tensor.matmul(ps2[0:100, 0:128], w2, h1[:])
                h2 = mpool.tile([100, 128], mm_dt, tag="h2")
                nc.scalar.activation(h2[:], ps2[0:100, 0:128], Relu, bias=b2)

                ps3 = ppool.tile([128, P_CORE], f32, tag="ps")
                nc.tensor.matmul(ps3[0:100, 0:128], w3, h2[:])
                h3 = mpool.tile([100, 128], mm_dt, tag="h3")
                nc.scalar.activation(h3[:], ps3[0:100, 0:128], Relu, bias=b3)

                ps4 = ppool.tile([128, P_CORE], f32, tag="ps")
                nc.tensor.matmul(ps4[0:128, 0:128], w4, h3[:])
                dg = cpool.tile([128, 128], mm_dt, tag=f"diag{c}")
                nc.scalar.activation(dg[:], ps4[0:128, 0:128], Ident, bias=b4)
                diag_g.append(dg)

            # ---- main: out[g] = exp(diag_g.T @ ET_u + lnN), one store per g ----
            for g in range(N_GRP):
                ps = ppool.tile([128, P_CORE], f32, tag="ps")
                for off in range(0, P_CORE, SUB):
                    w = min(SUB, P_CORE - off)
                    nc.tensor.matmul(
                        ps[:, off : off + w],
                        diag_g[g][:],
                        et_t[:, off : off + w],
                    )
                stage = spool.tile([128, P_CORE], f32, tag="stage")
                nc.scalar.activation(stage[:], ps[:], Exp, bias=lnb)
                nc.sync.dma_start(
                    out_d[g * 128 : (g + 1) * 128, :],
                    stage[:],
                )

    nc.compile()
    return nc


def _get_cached():
    key = ("nc", MODE)
    if key not in _CACHE:
        _CACHE[key] = _build_program(MODE)
    if "consts" not in _CACHE:
        _CACHE["consts"] = _build_constants()
    return (_CACHE[key],) + _CACHE["consts"]


def _make_in_maps(phi, W1, b1, W2, b2, W3, b3, W4, b4, ET):
    # fold the input normalization into the first layer
    scale = (DPHI / SIG).astype(np.float32)
    shift = ((MIN_PHI - MU) / SIG).astype(np.float32)
    W1f = (np.asarray(W1, np.float32) * scale[:, None]).astype(np.float32)
    b1f = (np.asarray(b1, np.float32) + shift @ np.asarray(W1, np.float32)).astype(
        np.float32
    )

    pm = np.zeros((128, PM_COLS), np.float32)
    pm[0:2, _PM_PHT] = np.asarray(phi, np.float32).T
    pm[0:2, _PM_W1] = W1f
    pm[0:100, _PM_W2] = np.asarray(W2, np.float32)
    pm[0:100, _PM_W3] = np.asarray(W3, np.float32)
    pm[0:100, _PM_W4] = np.asarray(W4, np.float32)
    pb = np.zeros((128, PB_COLS), np.float32)
    pb[0:100, 0] = np.asarray(b1f, np.float32)
    pb[0:100, 1] = np.asarray(b2, np.float32)
    pb[0:100, 2] = np.asarray(b3, np.float32)
    pb[0:128, 3] = np.asarray(b4, np.float32)
    pb[:, 4] = np.log(np.float64(NORM))

    common = {"pm": pm, "pb": pb}
    in_maps = []
    for c in range(N_CORES):
        m = dict(common)
        m["et"] = np.ascontiguousarray(ET[:, c * P_CORE : (c + 1) * P_CORE])
        in_maps.append(m)
    return in_maps


def kernel(phi, W1, b1, W2, b2, W3, b3, W4, b4):
    from concourse.bass_utils import run_bass_kernel_spmd

    nc, ET, IDX = _get_cached()
    in_maps = _make_in_maps(phi, W1, b1, W2, b2, W3, b3, W4, b4, ET)
    res = run_bass_kernel_spmd(nc, in_maps, core_ids=list(range(N_CORES)))
    uniq = np.concatenate([r["out"] for r in res.results], axis=1)  # (512, 8448)
    full = np.take(uniq, IDX, axis=1)  # (512, 65536) constant-gather replication
    return np.ascontiguousarray(full.reshape(B, 256, 256))


# revision 24
# speedup vs baseline: 2.9549x; 1.1140x over previous
"""Trainium2 Bass kernel for the CMB power-spectrum emulator problem.

Math: a 4-layer MLP maps phi (512,2) -> diag (128 knots, 512 ch); a natural
cubic spline through the 128 knots is evaluated on a constant 256x256
isotropic-frequency grid, then exp(.)*NORM.

Two structural collapses, both input-independent:
 1. The spline is linear in the knot values, so the whole spline stage is
    one constant matrix E:  out = exp(E @ diag + ln NORM).
 2. The grid value wn_iso[i,j] depends only on (a,b) = sorted(|wn_i|,|wn_j|),
    an exact 8-fold dihedral symmetry: only 8385 of the 65536 grid points
    are distinct, and equal points produce bitwise-equal outputs. The device
    computes the 8385 unique points; the host replicates them with a
    constant gather.

Device work per core (unique-point sharding, 1056 points/core, 512 ch):
  MLP (tiny f32r matmuls, chunked by 128 channels) -> diag_g (128,128) x4
  per 128-channel group g: psum = diag_g.T @ ET_u  (TensorE, f32r)
                           stage = exp(psum+lnN)   (ScalarE LUT, ~2 ULP)
                           store (128, 1056) fp32
"""

import os

import numpy as np

B = 512
N_CORES = 8
N_UNIQ = 129 * 130 // 2       # 8385 distinct grid values
P_CORE = 1056                 # per-core unique points (8 x 1056 = 8448 padded)
P_PAD = N_CORES * P_CORE
NORM = 1.0 / 12661.0

MIN_PHI = np.array([50.0, 0.0075], np.float32)
DPHI = np.array([40.0, 0.0492], np.float32)
MU = np.array([70.0, 0.032], np.float32)
SIG = np.array([20.0, 0.025], np.float32)

# matmul dtype: "f32" (4 cyc/row, exact), "f32r" (1 cyc/row, ~19-bit mantissa)
MODE = os.environ.get("BASS_KERNEL_MODE", "f32r")

# packed-parameter column layout (partition dim x columns), matmul-dtype part
_PM_PHT = slice(0, 512)
_PM_W1 = slice(512, 612)
_PM_W2 = slice(612, 712)
_PM_W3 = slice(712, 812)
_PM_W4 = slice(812, 940)
PM_COLS = 940
PB_COLS = 5  # fp32 part: b1, b2, b3, b4, ln(NORM)

_CACHE = {}


def _spline_eval_matrix(wn_vals):
    """E (len(wn_vals), 128) fp32: natural-cubic-spline evaluation at wn_vals,
    linear in the 128 knot values (knots t_k = sqrt(2)*k in fp32)."""
    wn = (256.0 * np.fft.fftfreq(256, d=1.0)).reshape(256, 1)
    wn_iso = np.sqrt(wn**2 + wn.reshape(1, 256) ** 2)
    t32 = np.fft.fftshift(wn_iso).diagonal()[128:].astype(np.float32)  # (128,)

    n = 128
    t = t32.astype(np.float64)
    h = np.diff(t)
    A = np.diag(2.0 * (h[:-1] + h[1:])) + np.diag(h[1:-1], 1) + np.diag(h[1:-1], -1)
    D1 = np.zeros((n - 1, n))
    for i in range(n - 1):
        D1[i, i] = -1.0 / h[i]
        D1[i, i + 1] = 1.0 / h[i]
    D2 = 6.0 * (D1[1:] - D1[:-1])
    L = np.zeros((n, n))
    L[1:-1] = np.linalg.solve(A, D2)

    Sa = np.eye(n)[: n - 1]
    Sb = D1 - (h[:, None] / 6.0) * (2.0 * L[:-1] + L[1:])
    Sc = L[:-1] / 2.0
    Sd = (L[1:] - L[:-1]) / (6.0 * h[:, None])

    w32 = wn_vals.astype(np.float32)
    idx = np.clip(np.searchsorted(t32, w32, side="right") - 1, 0, n - 2)
    f = (w32 - t32[idx]).astype(np.float64)[:, None]
    E = Sa[idx] + f * (Sb[idx] + f * (Sc[idx] + f * Sd[idx]))
    return E.astype(np.float32)


def _build_constants():
    """ET_u (128, P_PAD) fp32 for the unique points, and IDX (65536,) int32
    mapping each full-grid point to its unique-point column."""
    k = np.arange(256)
    absw = np.minimum(k, 256 - k)  # |wn_i|, with |wn_0| = 0, |wn_128| = 128
    ai = np.minimum(absw[:, None], absw[None, :])
    bi = np.maximum(absw[:, None], absw[None, :])
    uid = (bi * (bi + 1)) // 2 + ai  # (256,256) in [0, N_UNIQ)

    bs = np.concatenate([np.full(b + 1, b) for b in range(129)])  # uid -> b
    as_ = np.concatenate([np.arange(b + 1) for b in range(129)])  # uid -> a
    wn_vals = np.sqrt((as_.astype(np.float64)) ** 2 + (bs.astype(np.float64)) ** 2)

    E = _spline_eval_matrix(wn_vals)  # (8385, 128)
    ET = np.zeros((128, P_PAD), np.float32)
    ET[:, :N_UNIQ] = E.T
    return np.ascontiguousarray(ET), uid.ravel().astype(np.int32)


def _build_program(mode):
    import concourse.bass as bass
    import concourse.bacc as bacc
    import concourse.mybir as mybir
    from concourse import tile

    f32 = mybir.dt.float32
    mm_dt = {"f32r": mybir.dt.float32r, "f32": f32}[mode]
    nc = bacc.Bacc("TRN2", target_bir_lowering=False, debug=False)

    pm_d = nc.dram_tensor("pm", [128, PM_COLS], mm_dt, kind="ExternalInput")
    pb_d = nc.dram_tensor("pb", [128, PB_COLS], f32, kind="ExternalInput")
    et_d = nc.dram_tensor("et", [128, P_CORE], mm_dt, kind="ExternalInput")
    out_d = nc.dram_tensor("out", [B, P_CORE], f32, kind="ExternalOutput")

    Relu = mybir.ActivationFunctionType.Relu
    Ident = mybir.ActivationFunctionType.Identity
    Exp = mybir.ActivationFunctionType.Exp

    N_GRP = 4
    SUB = 512  # matmul free chunk (PSUM bank)
    N_WARM = 3  # PE warm-up matmuls during the load window (HAM un-throttle)

    with tile.TileContext(nc) as tc:
        with (
            tc.tile_pool(name="const", bufs=1) as cpool,
            tc.tile_pool(name="mlp", bufs=2) as mpool,
            tc.tile_pool(name="stage", bufs=2) as spool,
            tc.tile_pool(name="psum", bufs=2, space=bass.MemorySpace.PSUM) as ppool,
            tc.tile_pool(name="mpsum", bufs=2, space=bass.MemorySpace.PSUM) as mps,
        ):
            # ---- loads on the idle SP ring: params first, then ET ----
            pm_t = cpool.tile([128, PM_COLS], mm_dt, tag="pm")
            nc.sync.dma_start(pm_t[:], pm_d[:])
            pb_t = cpool.tile([128, PB_COLS], f32, tag="pb")
            nc.sync.dma_start(pb_t[:], pb_d[:])
            et_t = cpool.tile([128, P_CORE], mm_dt, tag="et")
            nc.sync.dma_start(et_t[:], et_d[:])

            pht = pm_t[0:2, _PM_PHT]
            w1 = pm_t[0:2, _PM_W1]
            w2 = pm_t[0:100, _PM_W2]
            w3 = pm_t[0:100, _PM_W3]
            w4 = pm_t[0:100, _PM_W4]
            b1 = pb_t[0:100, 0:1]
            b2 = pb_t[0:100, 1:2]
            b3 = pb_t[0:100, 2:3]
            b4 = pb_t[0:128, 3:4]
            lnb = pb_t[0:128, 4:5]

            # ---- PE warm-up: garbage matmuls fill the DMA-wait window so the
            # HAM clock gate opens (1.2 -> 2.4 GHz) before the real work ----
            wsrc = cpool.tile([128, SUB], f32, tag="wsrc")
            nc.gpsimd.memset(wsrc[:], 0.0)
            for _ in range(N_WARM):
                wp = mps.tile([128, SUB], f32, tag="mps")
                nc.tensor.matmul(wp[:], wsrc[0:128, 0:128], wsrc[:])

            # ---- MLP, full 512-channel width, PE<->ACT ping-pong ----
            ps1 = mps.tile([128, SUB], f32, tag="mps")
            nc.tensor.matmul(ps1[0:100, :], w1, pht)
            h1 = mpool.tile([100, B], mm_dt, tag="h1")
            nc.scalar.activation(h1[:], ps1[0:100, :], Relu, bias=b1)

            ps2 = mps.tile([128, SUB], f32, tag="mps")
            nc.tensor.matmul(ps2[0:100, :], w2, h1[:])
            h2 = mpool.tile([100, B], mm_dt, tag="h2")
            nc.scalar.activation(h2[:], ps2[0:100, :], Relu, bias=b2)

            ps3 = mps.tile([128, SUB], f32, tag="mps")
            nc.tensor.matmul(ps3[0:100, :], w3, h2[:])
            h3 = mpool.tile([100, B], mm_dt, tag="h3")
            nc.scalar.activation(h3[:], ps3[0:100, :], Relu, bias=b3)

            ps4 = mps.tile([128, SUB], f32, tag="mps")
            nc.tensor.matmul(ps4[:], w4, h3[:])
            diag = mpool.tile([128, B], mm_dt, tag="diag")
            nc.scalar.activation(diag[:], ps4[:], Ident, bias=b4)

            # ---- main: out[g] = exp(diag_g.T @ ET_u + lnN), one store per g ----
            for g in range(N_GRP):
                ps = ppool.tile([128, P_CORE], f32, tag="ps")
                for off in range(0, P_CORE, SUB):
                    w = min(SUB, P_CORE - off)
                    nc.tensor.matmul(
                        ps[:, off : off + w],
                        diag[:, g * 128 : (g + 1) * 128],
                        et_t[:, off : off + w],
                    )
                stage = spool.tile([128, P_CORE], f32, tag="stage")
                nc.scalar.activation(stage[:], ps[:], Exp, bias=lnb)
                nc.sync.dma_start(
                    out_d[g * 128 : (g + 1) * 128, :],
                    stage[:],
                )

    nc.compile()
    return nc


def _get_cached():
    key = ("nc", MODE)
    if key not in _CACHE:
        _CACHE[key] = _build_program(MODE)
    if "consts" not in _CACHE:
        _CACHE["consts"] = _build_constants()
    return (_CACHE[key],) + _CACHE["consts"]


def _make_in_maps(phi, W1, b1, W2, b2, W3, b3, W4, b4, ET):
    # fold the input normalization into the first layer
    scale = (DPHI / SIG).astype(np.float32)
    shift = ((MIN_PHI - MU) / SIG).astype(np.float32)
    W1f = (np.asarray(W1, np.float32) * scale[:, None]).astype(np.float32)
    b1f = (np.asarray(b1, np.float32) + shift @ np.asarray(W1, np.float32)).astype(
        np.float32
    )

    pm = np.zeros((128, PM_COLS), np.float32)
    pm[0:2, _PM_PHT] = np.asarray(phi, np.float32).T
    pm[0:2, _PM_W1] = W1f
    pm[0:100, _PM_W2] = np.asarray(W2, np.float32)
    pm[0:100, _PM_W3] = np.asarray(W3, np.float32)
    pm[0:100, _PM_W4] = np.asarray(W4, np.float32)
    pb = np.zeros((128, PB_COLS), np.float32)
    pb[0:100, 0] = np.asarray(b1f, np.float32)
    pb[0:100, 1] = np.asarray(b2, np.float32)
    pb[0:100, 2] = np.asarray(b3, np.float32)
    pb[0:128, 3] = np.asarray(b4, np.float32)
    pb[:, 4] = np.log(np.float64(NORM))

    common = {"pm": pm, "pb": pb}
    in_maps = []
    for c in range(N_CORES):
        m = dict(common)
        m["et"] = np.ascontiguousarray(ET[:, c * P_CORE : (c + 1) * P_CORE])
        in_maps.append(m)
    return in_maps


def kernel(phi, W1, b1, W2, b2, W3, b3, W4, b4):
    from concourse.bass_utils import run_bass_kernel_spmd

    nc, ET, IDX = _get_cached()
    in_maps = _make_in_maps(phi, W1, b1, W2, b2, W3, b3, W4, b4, ET)
    res = run_bass_kernel_spmd(nc, in_maps, core_ids=list(range(N_CORES)))
    uniq = np.concatenate([r["out"] for r in res.results], axis=1)  # (512, 8448)
    full = np.take(uniq, IDX, axis=1)  # (512, 65536) constant-gather replication
    return np.ascontiguousarray(full.reshape(B, 256, 256))


# revision 27
# speedup vs baseline: 3.5422x; 1.1988x over previous
"""Trainium2 Bass kernel for the CMB power-spectrum emulator problem.

Math: a 4-layer MLP maps phi (512,2) -> diag (128 knots, 512 ch); a natural
cubic spline through the 128 knots is evaluated on a constant 256x256
isotropic-frequency grid, then exp(.)*NORM.

Two structural collapses, both input-independent:
 1. The spline is linear in the knot values, so the whole spline stage is
    one constant matrix E:  out = exp(E @ diag + ln NORM).
 2. The grid value wn_iso[i,j] depends only on (a,b) = sorted(|wn_i|,|wn_j|),
    an exact 8-fold dihedral symmetry: only 8385 of the 65536 grid points
    are distinct, and equal points produce bitwise-equal outputs. The device
    computes the 8385 unique points; the host replicates them with a
    constant gather.

Device work per core (unique-point sharding, 1056 points/core, 512 ch):
  MLP (tiny f32r matmuls, chunked by 128 channels) -> diag_g (128,128) x4
  per 128-channel group g: psum = diag_g.T @ ET_u  (TensorE, f32r)
                           stage = exp(psum+lnN)   (ScalarE LUT, ~2 ULP)
                           store (128, 1056) fp32
"""

import os

import numpy as np

B = 512
N_CORES = 8
N_UNIQ = 129 * 130 // 2       # 8385 distinct grid values
P_CORE = 1056                 # per-core unique points (8 x 1056 = 8448 padded)
P_PAD = N_CORES * P_CORE
NORM = 1.0 / 12661.0

MIN_PHI = np.array([50.0, 0.0075], np.float32)
DPHI = np.array([40.0, 0.0492], np.float32)
MU = np.array([70.0, 0.032], np.float32)
SIG = np.array([20.0, 0.025], np.float32)

# matmul dtype: "f32" (4 cyc/row, exact), "f32r" (1 cyc/row, ~19-bit mantissa)
MODE = os.environ.get("BASS_KERNEL_MODE", "f32r")

# packed-parameter column layout (partition dim x columns), matmul-dtype part
_PM_PHT = slice(0, 512)
_PM_W1 = slice(512, 612)
_PM_W2 = slice(612, 712)
_PM_W3 = slice(712, 812)
_PM_W4 = slice(812, 940)
PM_COLS = 940
PB_COLS = 5  # fp32 part: b1, b2, b3, b4, ln(NORM)

_CACHE = {}


def _spline_eval_matrix(wn_vals):
    """E (len(wn_vals), 128) fp32: natural-cubic-spline evaluation at wn_vals,
    linear in the 128 knot values (knots t_k = sqrt(2)*k in fp32)."""
    wn = (256.0 * np.fft.fftfreq(256, d=1.0)).reshape(256, 1)
    wn_iso = np.sqrt(wn**2 + wn.reshape(1, 256) ** 2)
    t32 = np.fft.fftshift(wn_iso).diagonal()[128:].astype(np.float32)  # (128,)

    n = 128
    t = t32.astype(np.float64)
    h = np.diff(t)
    A = np.diag(2.0 * (h[:-1] + h[1:])) + np.diag(h[1:-1], 1) + np.diag(h[1:-1], -1)
    D1 = np.zeros((n - 1, n))
    for i in range(n - 1):
        D1[i, i] = -1.0 / h[i]
        D1[i, i + 1] = 1.0 / h[i]
    D2 = 6.0 * (D1[1:] - D1[:-1])
    L = np.zeros((n, n))
    L[1:-1] = np.linalg.solve(A, D2)

    Sa = np.eye(n)[: n - 1]
    Sb = D1 - (h[:, None] / 6.0) * (2.0 * L[:-1] + L[1:])
    Sc = L[:-1] / 2.0
    Sd = (L[1:] - L[:-1]) / (6.0 * h[:, None])

    w32 = wn_vals.astype(np.float32)
    idx = np.clip(np.searchsorted(t32, w32, side="right") - 1, 0, n - 2)
    f = (w32 - t32[idx]).astype(np.float64)[:, None]
    E = Sa[idx] + f * (Sb[idx] + f * (Sc[idx] + f * Sd[idx]))
    return E.astype(np.float32)


def _build_constants():
    """ET_u (128, P_PAD) fp32 for the unique points, and IDX (65536,) int32
    mapping each full-grid point to its unique-point column."""
    k = np.arange(256)
    absw = np.minimum(k, 256 - k)  # |wn_i|, with |wn_0| = 0, |wn_128| = 128
    ai = np.minimum(absw[:, None], absw[None, :])
    bi = np.maximum(absw[:, None], absw[None, :])
    uid = (bi * (bi + 1)) // 2 + ai  # (256,256) in [0, N_UNIQ)

    bs = np.concatenate([np.full(b + 1, b) for b in range(129)])  # uid -> b
    as_ = np.concatenate([np.arange(b + 1) for b in range(129)])  # uid -> a
    wn_vals = np.sqrt((as_.astype(np.float64)) ** 2 + (bs.astype(np.float64)) ** 2)

    E = _spline_eval_matrix(wn_vals)  # (8385, 128)
    ET = np.zeros((128, P_PAD), np.float32)
    ET[:, :N_UNIQ] = E.T
    return np.ascontiguousarray(ET), uid.ravel().astype(np.int32)


def _build_program(mode):
    import concourse.bass as bass
    import concourse.bacc as bacc
    import concourse.mybir as mybir
    from concourse import tile

    f32 = mybir.dt.float32
    mm_dt = {"f32r": mybir.dt.float32r, "f32": f32}[mode]
    nc = bacc.Bacc("TRN2", target_bir_lowering=False, debug=False)

    pm_d = nc.dram_tensor("pm", [128, PM_COLS], mm_dt, kind="ExternalInput")
    pb_d = nc.dram_tensor("pb", [128, PB_COLS], f32, kind="ExternalInput")
    et_d = nc.dram_tensor("et", [128, P_CORE], mm_dt, kind="ExternalInput")
    out_d = nc.dram_tensor("out", [B, P_CORE], f32, kind="ExternalOutput")

    Relu = mybir.ActivationFunctionType.Relu
    Ident = mybir.ActivationFunctionType.Identity
    Exp = mybir.ActivationFunctionType.Exp

    N_GRP = 4
    SUB = 512  # matmul free chunk (PSUM bank)
    N_WARM = 2  # PE warm-up matmuls during the load window (HAM un-throttle)

    with tile.TileContext(nc) as tc:
        with (
            tc.tile_pool(name="const", bufs=1) as cpool,
            tc.tile_pool(name="mlp", bufs=2) as mpool,
            tc.tile_pool(name="stage", bufs=4) as spool,
            tc.tile_pool(name="psum", bufs=2, space=bass.MemorySpace.PSUM) as ppool,
            tc.tile_pool(name="mpsum", bufs=2, space=bass.MemorySpace.PSUM) as mps,
        ):
            # ---- loads on the idle SP ring: params first, then ET ----
            pm_t = cpool.tile([128, PM_COLS], mm_dt, tag="pm")
            nc.sync.dma_start(pm_t[:], pm_d[:])
            pb_t = cpool.tile([128, PB_COLS], f32, tag="pb")
            nc.sync.dma_start(pb_t[:], pb_d[:])
            et_t = cpool.tile([128, P_CORE], mm_dt, tag="et")
            nc.sync.dma_start(et_t[:], et_d[:])

            pht = pm_t[0:2, _PM_PHT]
            w1 = pm_t[0:2, _PM_W1]
            w2 = pm_t[0:100, _PM_W2]
            w3 = pm_t[0:100, _PM_W3]
            w4 = pm_t[0:100, _PM_W4]
            b1 = pb_t[0:100, 0:1]
            b2 = pb_t[0:100, 1:2]
            b3 = pb_t[0:100, 2:3]
            b4 = pb_t[0:128, 3:4]
            lnb = pb_t[0:128, 4:5]

            # ---- PE warm-up: garbage matmuls fill the DMA-wait window so the
            # HAM clock gate opens (1.2 -> 2.4 GHz) before the real work ----
            wsrc = cpool.tile([128, SUB], f32, tag="wsrc")
            nc.gpsimd.memset(wsrc[:], 0.0)
            for _ in range(N_WARM):
                wp = mps.tile([128, SUB], f32, tag="mps")
                nc.tensor.matmul(wp[:], wsrc[0:128, 0:128], wsrc[:])

            # ---- MLP, two interleaved 256-wide chains (hides sem latency) ----
            HB = B // 2
            diag = mpool.tile([128, B], mm_dt, tag="diag")
            hs = {}
            for lyr, (wt, bt, act, win, wout) in enumerate(
                [
                    (w1, b1, Relu, 2, 100),
                    (w2, b2, Relu, 100, 100),
                    (w3, b3, Relu, 100, 100),
                    (w4, b4, Ident, 100, 128),
                ]
            ):
                for c in range(2):
                    cs = slice(c * HB, (c + 1) * HB)
                    src = pht[:, cs] if lyr == 0 else hs[c][:]
                    ps = mps.tile([128, SUB], f32, tag="mps")
                    nc.tensor.matmul(ps[0:wout, 0:HB], wt, src)
                    if lyr < 3:
                        h = mpool.tile([100, HB], mm_dt, tag=f"h{lyr}{c}")
                        nc.scalar.activation(h[:], ps[0:wout, 0:HB], act, bias=bt)
                        hs[c] = h
                    else:
                        nc.scalar.activation(
                            diag[:, cs], ps[0:wout, 0:HB], act, bias=bt
                        )

            # ---- main: out[g] = exp(diag_g.T @ ET_u + lnN), one store per g ----
            for g in range(N_GRP):
                ps = ppool.tile([128, P_CORE], f32, tag="ps")
                for off in range(0, P_CORE, SUB):
                    w = min(SUB, P_CORE - off)
                    nc.tensor.matmul(
                        ps[:, off : off + w],
                        diag[:, g * 128 : (g + 1) * 128],
                        et_t[:, off : off + w],
                    )
                stage = spool.tile([128, P_CORE], f32, tag="stage")
                nc.scalar.activation(stage[:], ps[:], Exp, bias=lnb)
                nc.sync.dma_start(
                    out_d[g * 128 : (g + 1) * 128, :],
                    stage[:],
                )

    nc.compile()
    return nc


def _get_cached():
    key = ("nc", MODE)
    if key not in _CACHE:
        _CACHE[key] = _build_program(MODE)
    if "consts" not in _CACHE:
        _CACHE["consts"] = _build_constants()
    return (_CACHE[key],) + _CACHE["consts"]


def _make_in_maps(phi, W1, b1, W2, b2, W3, b3, W4, b4, ET):
    # fold the input normalization into the first layer
    scale = (DPHI / SIG).astype(np.float32)
    shift = ((MIN_PHI - MU) / SIG).astype(np.float32)
    W1f = (np.asarray(W1, np.float32) * scale[:, None]).astype(np.float32)
    b1f = (np.asarray(b1, np.float32) + shift @ np.asarray(W1, np.float32)).astype(
        np.float32
    )

    pm = np.zeros((128, PM_COLS), np.float32)
    pm[0:2, _PM_PHT] = np.asarray(phi, np.float32).T
    pm[0:2, _PM_W1] = W1f
    pm[0:100, _PM_W2] = np.asarray(W2, np.float32)
    pm[0:100, _PM_W3] = np.asarray(W3, np.float32)
    pm[0:100, _PM_W4] = np.asarray(W4, np.float32)
    pb = np.zeros((128, PB_COLS), np.float32)
    pb[0:100, 0] = np.asarray(b1f, np.float32)
    pb[0:100, 1] = np.asarray(b2, np.float32)
    pb[0:100, 2] = np.asarray(b3, np.float32)
    pb[0:128, 3] = np.asarray(b4, np.float32)
    pb[:, 4] = np.log(np.float64(NORM))

    common = {"pm": pm, "pb": pb}
    in_maps = []
    for c in range(N_CORES):
        m = dict(common)
        m["et"] = np.ascontiguousarray(ET[:, c * P_CORE : (c + 1) * P_CORE])
        in_maps.append(m)
    return in_maps


def kernel(phi, W1, b1, W2, b2, W3, b3, W4, b4):
    from concourse.bass_utils import run_bass_kernel_spmd

    nc, ET, IDX = _get_cached()
    in_maps = _make_in_maps(phi, W1, b1, W2, b2, W3, b3, W4, b4, ET)
    res = run_bass_kernel_spmd(nc, in_maps, core_ids=list(range(N_CORES)))
    uniq = np.concatenate([r["out"] for r in res.results], axis=1)  # (512, 8448)
    full = np.take(uniq, IDX, axis=1)  # (512, 65536) constant-gather replication
    return np.ascontiguousarray(full.reshape(B, 256, 256))
